# revision 6
# baseline (speedup 1.0000x reference)
"""Trainium2 Bass kernel for GPT2 sparse-attention decode step (B=8, T=8192, H=16, HD=64).

Contract: kernel(**inputs) takes FULL unsharded inputs, returns the FULL output
tuple (y[8,1024], k_step[8,16,64], v_step[8,16,64]) matching reference().

Strategy (memory-regime): only cache positions t <= idx[b] matter. The tiny
qkv / output projections and the flash-style combine run on host; the 8
NeuronCores do the memory-dominant sweep over the valid KV prefix. Each
batch's prefix [0, idx[b]) is split contiguously into 8 equal per-core chunks
(balanced load); the freshly-written token at t=idx[b] is folded in on host.

Per (core, batch) device program:
  scores[h,t] = sum_d q[b,h,d]*K[t,h,d]*scale  -- PE matmul with block-diag q
                (contraction over hd on partitions; K shipped pre-transposed
                 [hd, t] by host), accumulated over 8 hd-slices into PSUM.
  e = exp(scores)                              -- ScalarE, fused row-sum
  p^T via PE transpose (scores are provably small => no max-stabilization)
  o[h,:] = sum_t p[t,h] * V[t,:]               -- PE matmul, natural-layout V,
                diag 64-wide strips extracted on host.
Zero-padded tail positions contribute exp(0)=1 to the denominator and 0 to o;
the host subtracts the exact pad count.
"""
import math
import os
import sys

import numpy as np

for _p in ("/root/.axon_site/_ro/trn_rl_repo", "/opt/trn_rl_repo"):
    if os.path.isdir(_p) and _p not in sys.path:
        sys.path.append(_p)

B, T, D, H, HD = 8, 8192, 1024, 16, 64
NC = 8
SCALE = 1.0 / 8.0
CHUNK = 512
SCORES_F32R = True   # float32r matmul for q@K^T (1 cy/row vs 4 for fp32)
PV_F32R = True       # float32r matmul for p@V

_BUILD_CACHE: dict = {}


def _build_nc(n_bs):
    """Build the SPMD Bass program for per-batch per-core lengths n_bs."""
    import concourse.bacc as bacc
    import concourse.mybir as mybir
    import concourse.tile as tile
    from concourse.masks import make_identity

    f32 = mybir.dt.float32
    f32r = mybir.dt.float32r
    kdt = f32r if SCORES_F32R else f32
    vdt = f32r if PV_F32R else f32
    nc = bacc.Bacc("TRN2")

    kt_in = [nc.declare_dram_parameter(f"kt{b}", [8, 128, n_bs[b]], kdt, isOutput=False)
             for b in range(B)]
    v_in = [nc.declare_dram_parameter(f"v{b}", [n_bs[b], D], vdt, isOutput=False)
            for b in range(B)]
    q_in = nc.declare_dram_parameter("qblk", [128, B * 8 * 16], kdt, isOutput=False)
    o_out = nc.declare_dram_parameter("o", [B, 16, D], f32, isOutput=True)
    s_out = nc.declare_dram_parameter("s", [B, 16, 2], f32, isOutput=True)

    with tile.TileContext(nc) as tc:
        with (
            tc.tile_pool(name="singles", bufs=1) as singles,
            tc.tile_pool(name="kpool", bufs=3) as kpool,
            tc.tile_pool(name="vpool", bufs=3) as vpool,
            tc.tile_pool(name="epool", bufs=2) as epool,
            tc.tile_pool(name="ptpool", bufs=2) as ptpool,
            tc.tile_pool(name="spool", bufs=4) as spool,
            tc.tile_pool(name="opool", bufs=2) as opool,
            tc.tile_pool(name="psum_s", bufs=2, space="PSUM") as psum_s,
            tc.tile_pool(name="psum_t", bufs=2, space="PSUM") as psum_t,
            tc.tile_pool(name="psum_o", bufs=2, space="PSUM") as psum_o,
        ):
            qsb = singles.tile([128, B * 8 * 16], f32)
            nc.sync.dma_start(out=qsb[:], in_=q_in[:])
            ident = singles.tile([16, 16], f32)
            make_identity(nc, ident)

            for b in range(B):
                n = n_bs[b]
                nchunks = (n + CHUNK - 1) // CHUNK
                o_ps = psum_o.tile([16, D], f32)
                for ci in range(nchunks):
                    c0 = ci * CHUNK
                    w = min(CHUNK, n - c0)
                    njt = (w + 127) // 128

                    k_sb = kpool.tile([128, 8, CHUNK], f32, tag="k")
                    nc.sync.dma_start(
                        out=k_sb[:, :, :w],
                        in_=kt_in[b][:, :, c0:c0 + w].rearrange("s p t -> p s t"),
                    )

                    v_sb = vpool.tile([128, 4, D], f32, tag="v")
                    jfull = w // 128
                    rem = w % 128
                    if jfull:
                        nc.sync.dma_start(
                            out=v_sb[:, :jfull, :],
                            in_=v_in[b][c0:c0 + jfull * 128, :].rearrange(
                                "(j p) d -> p j d", p=128),
                        )
                    if rem:
                        nc.sync.dma_start(
                            out=v_sb[:rem, jfull, :],
                            in_=v_in[b][c0 + jfull * 128:c0 + w, :],
                        )

                    sc_ps = psum_s.tile([16, CHUNK], f32)
                    for s in range(8):
                        nc.tensor.matmul(
                            sc_ps[:, :w],
                            lhsT=qsb[:, (b * 8 + s) * 16:(b * 8 + s + 1) * 16],
                            rhs=k_sb[:, s, :w],
                            start=(s == 0),
                            stop=(s == 7),
                        )

                    e_sb = epool.tile([16, CHUNK], f32, tag="e")
                    s_sb = spool.tile([16, 1], f32, tag="s")
                    nc.scalar.activation(
                        out=e_sb[:, :w],
                        in_=sc_ps[:, :w],
                        func=mybir.ActivationFunctionType.Exp,
                        accum_out=s_sb[:],
                    )
                    nc.sync.dma_start(out=s_out[b, :, ci:ci + 1], in_=s_sb[:])

                    pt_ps = psum_t.tile([128, 64], f32)
                    pt_sb = ptpool.tile([128, 64], f32, tag="pt")
                    for j in range(njt):
                        wj = min(128, w - j * 128)
                        nc.tensor.transpose(
                            out=pt_ps[:wj, j * 16:(j + 1) * 16],
                            in_=e_sb[:, j * 128:j * 128 + wj],
                            identity=ident[:],
                        )
                        nc.vector.tensor_copy(
                            pt_sb[:wj, j * 16:(j + 1) * 16],
                            pt_ps[:wj, j * 16:(j + 1) * 16],
                        )

                    for j in range(njt):
                        wj = min(128, w - j * 128)
                        first = (ci == 0 and j == 0)
                        last = (ci == nchunks - 1 and j == njt - 1)
                        for half in range(2):
                            nc.tensor.matmul(
                                o_ps[:, half * 512:(half + 1) * 512],
                                lhsT=pt_sb[:wj, j * 16:(j + 1) * 16],
                                rhs=v_sb[:wj, j, half * 512:(half + 1) * 512],
                                start=first,
                                stop=last,
                                skip_group_check=True,
                            )

                o_sb = opool.tile([16, D], f32, tag="o")
                nc.vector.tensor_copy(o_sb[:], o_ps[:])
                nc.sync.dma_start(out=o_out[b], in_=o_sb[:])

    nc.compile()
    return nc


def _host_prep(x, past_k, past_v, idx, W_attn, b_attn):
    x = np.ascontiguousarray(x, np.float32)
    qkv = x @ np.asarray(W_attn, np.float32) + np.asarray(b_attn, np.float32)
    q, k_step, v_step = np.split(qkv, 3, axis=-1)
    q = q.reshape(B, H, HD)
    k_step = np.ascontiguousarray(k_step.reshape(B, H, HD))
    v_step = np.ascontiguousarray(v_step.reshape(B, H, HD))

    idx = np.asarray(idx).astype(np.int64)
    P = idx.copy()                                  # past length (t < idx)
    n_bs = tuple(int(max(1, (p + NC - 1) // NC)) for p in P)

    qblk = np.zeros((B, 8, 128, 16), np.float32)
    hs, ds_ = np.meshgrid(np.arange(H), np.arange(HD), indexing="ij")
    flat = hs * HD + ds_
    s_i, p_i = flat // 128, flat % 128
    for b in range(B):
        qblk[b, s_i, p_i, hs] = q[b] * SCALE
    qblk = np.ascontiguousarray(qblk.transpose(2, 0, 1, 3).reshape(128, B * 8 * 16))

    pk = np.asarray(past_k, np.float32).reshape(B, T, D)
    pv = np.asarray(past_v, np.float32).reshape(B, T, D)

    in_maps = []
    valid = np.zeros((NC, B), np.int64)
    for c in range(NC):
        m = {"qblk": qblk}
        for b in range(B):
            n = n_bs[b]
            lo = c * n
            hi = min((c + 1) * n, int(P[b]))
            v = max(0, hi - lo)
            valid[c, b] = v
            kc = np.zeros((n, D), np.float32)
            vc = np.zeros((n, D), np.float32)
            if v > 0:
                kc[:v] = pk[b, lo:lo + v]
                vc[:v] = pv[b, lo:lo + v]
            m[f"kt{b}"] = np.ascontiguousarray(kc.T).reshape(8, 128, n)
            m[f"v{b}"] = vc
        in_maps.append(m)

    meta = dict(q=q, k_step=k_step, v_step=v_step, n_bs=n_bs, valid=valid)
    return in_maps, meta


def _host_combine(results, meta, W_proj, b_proj):
    q, k_step, v_step = meta["q"], meta["k_step"], meta["v_step"]
    n_bs, valid = meta["n_bs"], meta["valid"]
    O = np.zeros((B, H, HD), np.float64)
    S = np.zeros((B, H), np.float64)
    hh = np.arange(H)
    nch = np.array([(n + CHUNK - 1) // CHUNK for n in n_bs])
    for c in range(NC):
        o = results[c]["o"].astype(np.float64)      # [B, 16, 1024]
        s = results[c]["s"].astype(np.float64).copy()  # [B, 16, 2]
        for b in range(B):                          # mask never-written chunk cols
            s[b, :, nch[b]:] = 0.0
        o_diag = o.reshape(B, H, H, HD)[:, hh, hh, :]  # [B, H, HD]
        pads = np.array([n_bs[b] - valid[c, b] for b in range(B)], np.float64)
        S += s.sum(axis=2) - pads[:, None]
        O += o_diag
    s_new = np.einsum("bhd,bhd->bh", q.astype(np.float64),
                      k_step.astype(np.float64)) * SCALE
    e_new = np.exp(s_new)
    S += e_new
    O += e_new[:, :, None] * v_step.astype(np.float64)
    y_attn = (O / S[:, :, None]).reshape(B, D)
    y = y_attn @ np.asarray(W_proj, np.float64) + np.asarray(b_proj, np.float64)
    return y.astype(np.float32)


def _get_built(n_bs):
    if n_bs not in _BUILD_CACHE:
        _BUILD_CACHE[n_bs] = _build_nc(n_bs)
    return _BUILD_CACHE[n_bs]


def kernel(x, past_k, past_v, idx, W_attn, b_attn, W_proj, b_proj, _trace=False):
    from concourse.bass_utils import run_bass_kernel_spmd

    in_maps, meta = _host_prep(x, past_k, past_v, idx, W_attn, b_attn)
    nc = _get_built(meta["n_bs"])
    res = run_bass_kernel_spmd(nc, in_maps, core_ids=list(range(NC)), trace=_trace)
    y = _host_combine(res.results, meta, W_proj, b_proj)
    if _trace:
        kernel._last_results = res
    return (y, meta["k_step"], meta["v_step"])


# revision 12
# speedup vs baseline: 1.1926x; 1.1926x over previous
"""Trainium2 Bass kernel for GPT2 sparse-attention decode step (B=8, T=8192, H=16, HD=64).

Contract: kernel(**inputs) takes FULL unsharded inputs, returns the FULL output
tuple (y[8,1024], k_step[8,16,64], v_step[8,16,64]) matching reference().

Strategy (memory-regime): only cache positions t <= idx[b] matter. The tiny
qkv / output projections and the flash-style combine run on host; the 8
NeuronCores do the memory-dominant sweep over the valid KV prefix. Each
batch's prefix [0, idx[b]) is split contiguously into 8 equal per-core chunks
(balanced load); the freshly-written token at t=idx[b] is folded in on host.

Per (core, batch) device program:
  scores[h,t] = sum_d q[b,h,d]*K[t,h,d]*scale  -- PE matmul with block-diag q
                (contraction over hd on partitions; K shipped pre-transposed
                 [hd, t] by host), accumulated over 8 hd-slices into PSUM.
  e = exp(scores)                              -- ScalarE, fused row-sum
  p^T via PE transpose (scores are provably small => no max-stabilization)
  o[h,:] = sum_t p[t,h] * V[t,:]               -- PE matmul, natural-layout V,
                diag 64-wide strips extracted on host.
Zero-padded tail positions contribute exp(0)=1 to the denominator and 0 to o;
the host subtracts the exact pad count.
"""
import math
import os
import sys

import numpy as np

for _p in ("/root/.axon_site/_ro/trn_rl_repo", "/opt/trn_rl_repo"):
    if os.path.isdir(_p) and _p not in sys.path:
        sys.path.append(_p)

B, T, D, H, HD = 8, 8192, 1024, 16, 64
NC = 8
SCALE = 1.0 / 8.0
CHUNK = 512
SCORES_F32R = True   # float32r matmul for q@K^T (1 cy/row vs 4 for fp32)
PV_F32R = True       # float32r matmul for p@V

_BUILD_CACHE: dict = {}


def _build_nc(n_bs):
    """Build the SPMD Bass program for per-batch per-core lengths n_bs."""
    import concourse.bacc as bacc
    import concourse.mybir as mybir
    import concourse.tile as tile
    from concourse.masks import make_identity

    f32 = mybir.dt.float32
    f32r = mybir.dt.float32r
    kdt = f32r if SCORES_F32R else f32
    vdt = f32r if PV_F32R else f32

    nc = bacc.Bacc("TRN2")

    kt_in = [nc.declare_dram_parameter(f"kt{b}", [8, 128, n_bs[b]], kdt, isOutput=False)
             for b in range(B)]
    v_in = [nc.declare_dram_parameter(f"v{b}", [n_bs[b], D], vdt, isOutput=False)
            for b in range(B)]
    q_in = nc.declare_dram_parameter("qblk", [128, B * 8 * 16], kdt, isOutput=False)
    o_out = nc.declare_dram_parameter("o", [B, 16, D], f32, isOutput=True)
    s_out = nc.declare_dram_parameter("s", [B, 16, 2], f32, isOutput=True)

    with tile.TileContext(nc) as tc:
        with (
            tc.tile_pool(name="singles", bufs=1) as singles,
            tc.tile_pool(name="kpool", bufs=3) as kpool,
            tc.tile_pool(name="vpool", bufs=3) as vpool,
            tc.tile_pool(name="epool", bufs=2) as epool,
            tc.tile_pool(name="ptpool", bufs=2) as ptpool,
            tc.tile_pool(name="spool", bufs=4) as spool,
            tc.tile_pool(name="opool", bufs=2) as opool,
            tc.tile_pool(name="psum_s", bufs=2, space="PSUM") as psum_s,
            tc.tile_pool(name="psum_t", bufs=2, space="PSUM") as psum_t,
            tc.tile_pool(name="psum_o", bufs=2, space="PSUM") as psum_o,
        ):
            qsb = singles.tile([128, B * 8 * 16], kdt)
            nc.sync.dma_start(out=qsb[:], in_=q_in[:])
            ident = singles.tile([16, 16], f32)
            make_identity(nc, ident)

            for b in range(B):
                n = n_bs[b]
                nchunks = (n + CHUNK - 1) // CHUNK
                o_ps = psum_o.tile([16, D], f32)
                for ci in range(nchunks):
                    c0 = ci * CHUNK
                    w = min(CHUNK, n - c0)
                    njt = (w + 127) // 128

                    k_sb = kpool.tile([128, 8, CHUNK], kdt, tag="k")
                    nc.sync.dma_start(
                        out=k_sb[:, :, :w],
                        in_=kt_in[b][:, :, c0:c0 + w].rearrange("s p t -> p s t"),
                    )

                    v_sb = vpool.tile([128, 4, D], vdt, tag="v")
                    jfull = w // 128
                    rem = w % 128
                    if jfull:
                        nc.sync.dma_start(
                            out=v_sb[:, :jfull, :],
                            in_=v_in[b][c0:c0 + jfull * 128, :].rearrange(
                                "(j p) d -> p j d", p=128),
                        )
                    if rem:
                        nc.sync.dma_start(
                            out=v_sb[:rem, jfull, :],
                            in_=v_in[b][c0 + jfull * 128:c0 + w, :],
                        )

                    sc_ps = psum_s.tile([16, CHUNK], f32)
                    for s in range(8):
                        nc.tensor.matmul(
                            sc_ps[:, :w],
                            lhsT=qsb[:, (b * 8 + s) * 16:(b * 8 + s + 1) * 16],
                            rhs=k_sb[:, s, :w],
                            start=(s == 0),
                            stop=(s == 7),
                        )

                    e_sb = epool.tile([16, CHUNK], f32, tag="e")
                    s_sb = spool.tile([16, 1], f32, tag="s")
                    nc.scalar.activation(
                        out=e_sb[:, :w],
                        in_=sc_ps[:, :w],
                        func=mybir.ActivationFunctionType.Exp,
                        accum_out=s_sb[:],
                    )
                    nc.sync.dma_start(out=s_out[b, :, ci:ci + 1], in_=s_sb[:])

                    pt_ps = psum_t.tile([128, 64], f32)
                    pt_sb = ptpool.tile([128, 64], vdt, tag="pt")
                    for j in range(njt):
                        wj = min(128, w - j * 128)
                        nc.tensor.transpose(
                            out=pt_ps[:wj, j * 16:(j + 1) * 16],
                            in_=e_sb[:, j * 128:j * 128 + wj],
                            identity=ident[:],
                        )
                        nc.vector.tensor_copy(
                            pt_sb[:wj, j * 16:(j + 1) * 16],
                            pt_ps[:wj, j * 16:(j + 1) * 16],
                        )

                    for j in range(njt):
                        wj = min(128, w - j * 128)
                        first = (ci == 0 and j == 0)
                        last = (ci == nchunks - 1 and j == njt - 1)
                        for half in range(2):
                            nc.tensor.matmul(
                                o_ps[:, half * 512:(half + 1) * 512],
                                lhsT=pt_sb[:wj, j * 16:(j + 1) * 16],
                                rhs=v_sb[:wj, j, half * 512:(half + 1) * 512],
                                start=first,
                                stop=last,
                                skip_group_check=True,
                            )

                o_sb = opool.tile([16, D], f32, tag="o")
                nc.vector.tensor_copy(o_sb[:], o_ps[:])
                nc.sync.dma_start(out=o_out[b], in_=o_sb[:])

    nc.compile()
    return nc


def _host_prep(x, past_k, past_v, idx, W_attn, b_attn):
    x = np.ascontiguousarray(x, np.float32)
    qkv = x @ np.asarray(W_attn, np.float32) + np.asarray(b_attn, np.float32)
    q, k_step, v_step = np.split(qkv, 3, axis=-1)
    q = q.reshape(B, H, HD)
    k_step = np.ascontiguousarray(k_step.reshape(B, H, HD))
    v_step = np.ascontiguousarray(v_step.reshape(B, H, HD))

    idx = np.asarray(idx).astype(np.int64)
    P = idx.copy()                                  # past length (t < idx)
    # per-core per-batch positions, padded to a multiple of 4 (fp32r matmul
    # requires even moving-operand free dims; pads are exact zeros, fixed up
    # in the combine)
    n_bs = tuple(int(max(4, -(-((p + NC - 1) // NC) // 4) * 4)) for p in P)

    qblk = np.zeros((B, 8, 128, 16), np.float32)
    hs, ds_ = np.meshgrid(np.arange(H), np.arange(HD), indexing="ij")
    flat = hs * HD + ds_
    s_i, p_i = flat // 128, flat % 128
    for b in range(B):
        qblk[b, s_i, p_i, hs] = q[b] * SCALE
    qblk = np.ascontiguousarray(qblk.transpose(2, 0, 1, 3).reshape(128, B * 8 * 16))

    pk = np.asarray(past_k, np.float32).reshape(B, T, D)
    pv = np.asarray(past_v, np.float32).reshape(B, T, D)

    in_maps = []
    valid = np.zeros((NC, B), np.int64)
    for c in range(NC):
        m = {"qblk": qblk}
        for b in range(B):
            n = n_bs[b]
            lo = c * n
            hi = min((c + 1) * n, int(P[b]))
            v = max(0, hi - lo)
            valid[c, b] = v
            kc = np.zeros((n, D), np.float32)
            vc = np.zeros((n, D), np.float32)
            if v > 0:
                kc[:v] = pk[b, lo:lo + v]
                vc[:v] = pv[b, lo:lo + v]
            m[f"kt{b}"] = np.ascontiguousarray(kc.T).reshape(8, 128, n)
            m[f"v{b}"] = vc
        in_maps.append(m)

    meta = dict(q=q, k_step=k_step, v_step=v_step, n_bs=n_bs, valid=valid)
    return in_maps, meta


def _host_combine(results, meta, W_proj, b_proj):
    q, k_step, v_step = meta["q"], meta["k_step"], meta["v_step"]
    n_bs, valid = meta["n_bs"], meta["valid"]
    O = np.zeros((B, H, HD), np.float64)
    S = np.zeros((B, H), np.float64)
    hh = np.arange(H)
    nch = np.array([(n + CHUNK - 1) // CHUNK for n in n_bs])
    for c in range(NC):
        o = results[c]["o"].astype(np.float64)      # [B, 16, 1024]
        s = results[c]["s"].astype(np.float64).copy()  # [B, 16, 2]
        for b in range(B):                          # mask never-written chunk cols
            s[b, :, nch[b]:] = 0.0
        o_diag = o.reshape(B, H, H, HD)[:, hh, hh, :]  # [B, H, HD]
        pads = np.array([n_bs[b] - valid[c, b] for b in range(B)], np.float64)
        S += s.sum(axis=2) - pads[:, None]
        O += o_diag
    s_new = np.einsum("bhd,bhd->bh", q.astype(np.float64),
                      k_step.astype(np.float64)) * SCALE
    e_new = np.exp(s_new)
    S += e_new
    O += e_new[:, :, None] * v_step.astype(np.float64)
    y_attn = (O / S[:, :, None]).reshape(B, D)
    y = y_attn @ np.asarray(W_proj, np.float64) + np.asarray(b_proj, np.float64)
    return y.astype(np.float32)


def _get_built(n_bs):
    if n_bs not in _BUILD_CACHE:
        _BUILD_CACHE[n_bs] = _build_nc(n_bs)
    return _BUILD_CACHE[n_bs]


def kernel(x, past_k, past_v, idx, W_attn, b_attn, W_proj, b_proj, _trace=False):
    from concourse.bass_utils import run_bass_kernel_spmd

    in_maps, meta = _host_prep(x, past_k, past_v, idx, W_attn, b_attn)
    nc = _get_built(meta["n_bs"])
    res = run_bass_kernel_spmd(nc, in_maps, core_ids=list(range(NC)), trace=_trace)
    y = _host_combine(res.results, meta, W_proj, b_proj)
    if _trace:
        kernel._last_results = res
    return (y, meta["k_step"], meta["v_step"])


# revision 13
# speedup vs baseline: 1.2560x; 1.0532x over previous
"""Trainium2 Bass kernel for GPT2 sparse-attention decode step (B=8, T=8192, H=16, HD=64).

Contract: kernel(**inputs) takes FULL unsharded inputs, returns the FULL output
tuple (y[8,1024], k_step[8,16,64], v_step[8,16,64]) matching reference().

Strategy (memory-regime): only cache positions t <= idx[b] matter. The tiny
qkv / output projections and the flash-style combine run on host; the 8
NeuronCores do the memory-dominant sweep over the valid KV prefix. Each
batch's prefix [0, idx[b]) is split contiguously into 8 equal per-core chunks
(balanced load); the freshly-written token at t=idx[b] is folded in on host.

Per (core, batch) device program:
  scores[h,t] = sum_d q[b,h,d]*K[t,h,d]*scale  -- PE matmul with block-diag q
                (contraction over hd on partitions; K shipped pre-transposed
                 [hd, t] by host), accumulated over 8 hd-slices into PSUM.
  e = exp(scores)                              -- ScalarE, fused row-sum
  p^T via PE transpose (scores are provably small => no max-stabilization)
  o[h,:] = sum_t p[t,h] * V[t,:]               -- PE matmul, natural-layout V,
                diag 64-wide strips extracted on host.
Zero-padded tail positions contribute exp(0)=1 to the denominator and 0 to o;
the host subtracts the exact pad count.
"""
import math
import os
import sys

import numpy as np

for _p in ("/root/.axon_site/_ro/trn_rl_repo", "/opt/trn_rl_repo"):
    if os.path.isdir(_p) and _p not in sys.path:
        sys.path.append(_p)

B, T, D, H, HD = 8, 8192, 1024, 16, 64
NC = 8
SCALE = 1.0 / 8.0
CHUNK = 512
SCORES_F32R = True   # float32r matmul for q@K^T (1 cy/row vs 4 for fp32)
PV_F32R = True       # float32r matmul for p@V

_BUILD_CACHE: dict = {}


def _build_nc(n_bs):
    """Build the SPMD Bass program for per-batch per-core lengths n_bs."""
    import concourse.bacc as bacc
    import concourse.mybir as mybir
    import concourse.tile as tile
    from concourse.masks import make_identity

    f32 = mybir.dt.float32
    f32r = mybir.dt.float32r
    kdt = f32r if SCORES_F32R else f32
    vdt = f32r if PV_F32R else f32

    nc = bacc.Bacc("TRN2")

    kt_in = [nc.declare_dram_parameter(f"kt{b}", [8, 128, n_bs[b]], kdt, isOutput=False)
             for b in range(B)]
    v_in = [nc.declare_dram_parameter(f"v{b}", [n_bs[b], D], vdt, isOutput=False)
            for b in range(B)]
    q_in = nc.declare_dram_parameter("qblk", [128, B * 8 * 16], kdt, isOutput=False)
    o_out = nc.declare_dram_parameter("o", [B, 16, D], f32, isOutput=True)
    s_out = nc.declare_dram_parameter("s", [B, 16, 2], f32, isOutput=True)

    with tile.TileContext(nc) as tc:
        with (
            tc.tile_pool(name="singles", bufs=1) as singles,
            tc.tile_pool(name="kpool", bufs=3) as kpool,
            tc.tile_pool(name="vpool", bufs=3) as vpool,
            tc.tile_pool(name="epool", bufs=2) as epool,
            tc.tile_pool(name="ptpool", bufs=2) as ptpool,
            tc.tile_pool(name="spool", bufs=4) as spool,
            tc.tile_pool(name="opool", bufs=2) as opool,
            tc.tile_pool(name="psum_s", bufs=2, space="PSUM") as psum_s,
            tc.tile_pool(name="psum_t", bufs=2, space="PSUM") as psum_t,
            tc.tile_pool(name="psum_o", bufs=2, space="PSUM") as psum_o,
        ):
            qsb = singles.tile([128, B * 8 * 16], kdt)
            nc.sync.dma_start(out=qsb[:], in_=q_in[:])
            ident = singles.tile([16, 16], f32)
            make_identity(nc, ident)

            for b in range(B):
                n = n_bs[b]
                nchunks = (n + CHUNK - 1) // CHUNK
                o_ps = psum_o.tile([16, D], f32)
                for ci in range(nchunks):
                    c0 = ci * CHUNK
                    w = min(CHUNK, n - c0)
                    njt = (w + 127) // 128

                    k_sb = kpool.tile([128, 8, CHUNK], kdt, tag="k")
                    for g in range(2):
                        nc.sync.dma_start(
                            out=k_sb[:, g * 4:(g + 1) * 4, :w],
                            in_=kt_in[b][g * 4:(g + 1) * 4, :, c0:c0 + w].rearrange(
                                "s p t -> p s t"),
                        )

                    v_sb = vpool.tile([128, 4, D], vdt, tag="v")
                    jfull = w // 128
                    rem = w % 128
                    for j0 in range(jfull):
                        nc.sync.dma_start(
                            out=v_sb[:, j0:j0 + 1, :],
                            in_=v_in[b][c0 + j0 * 128:c0 + (j0 + 1) * 128, :].rearrange(
                                "(j p) d -> p j d", p=128),
                        )
                    if rem:
                        nc.sync.dma_start(
                            out=v_sb[:rem, jfull, :],
                            in_=v_in[b][c0 + jfull * 128:c0 + w, :],
                        )

                    sc_ps = psum_s.tile([16, CHUNK], f32)
                    for s in range(8):
                        nc.tensor.matmul(
                            sc_ps[:, :w],
                            lhsT=qsb[:, (b * 8 + s) * 16:(b * 8 + s + 1) * 16],
                            rhs=k_sb[:, s, :w],
                            start=(s == 0),
                            stop=(s == 7),
                        )

                    e_sb = epool.tile([16, CHUNK], f32, tag="e")
                    s_sb = spool.tile([16, 1], f32, tag="s")
                    nc.scalar.activation(
                        out=e_sb[:, :w],
                        in_=sc_ps[:, :w],
                        func=mybir.ActivationFunctionType.Exp,
                        accum_out=s_sb[:],
                    )
                    nc.sync.dma_start(out=s_out[b, :, ci:ci + 1], in_=s_sb[:])

                    pt_ps = psum_t.tile([128, 64], f32)
                    pt_sb = ptpool.tile([128, 64], vdt, tag="pt")
                    for j in range(njt):
                        wj = min(128, w - j * 128)
                        nc.tensor.transpose(
                            out=pt_ps[:wj, j * 16:(j + 1) * 16],
                            in_=e_sb[:, j * 128:j * 128 + wj],
                            identity=ident[:],
                        )
                        nc.vector.tensor_copy(
                            pt_sb[:wj, j * 16:(j + 1) * 16],
                            pt_ps[:wj, j * 16:(j + 1) * 16],
                        )

                    for j in range(njt):
                        wj = min(128, w - j * 128)
                        first = (ci == 0 and j == 0)
                        last = (ci == nchunks - 1 and j == njt - 1)
                        for half in range(2):
                            nc.tensor.matmul(
                                o_ps[:, half * 512:(half + 1) * 512],
                                lhsT=pt_sb[:wj, j * 16:(j + 1) * 16],
                                rhs=v_sb[:wj, j, half * 512:(half + 1) * 512],
                                start=first,
                                stop=last,
                                skip_group_check=True,
                            )

                o_sb = opool.tile([16, D], f32, tag="o")
                nc.vector.tensor_copy(o_sb[:], o_ps[:])
                nc.sync.dma_start(out=o_out[b], in_=o_sb[:])

    nc.compile()
    return nc


def _host_prep(x, past_k, past_v, idx, W_attn, b_attn):
    x = np.ascontiguousarray(x, np.float32)
    qkv = x @ np.asarray(W_attn, np.float32) + np.asarray(b_attn, np.float32)
    q, k_step, v_step = np.split(qkv, 3, axis=-1)
    q = q.reshape(B, H, HD)
    k_step = np.ascontiguousarray(k_step.reshape(B, H, HD))
    v_step = np.ascontiguousarray(v_step.reshape(B, H, HD))

    idx = np.asarray(idx).astype(np.int64)
    P = idx.copy()                                  # past length (t < idx)
    # per-core per-batch positions, padded to a multiple of 4 (fp32r matmul
    # requires even moving-operand free dims; pads are exact zeros, fixed up
    # in the combine)
    n_bs = tuple(int(max(4, -(-((p + NC - 1) // NC) // 4) * 4)) for p in P)

    qblk = np.zeros((B, 8, 128, 16), np.float32)
    hs, ds_ = np.meshgrid(np.arange(H), np.arange(HD), indexing="ij")
    flat = hs * HD + ds_
    s_i, p_i = flat // 128, flat % 128
    for b in range(B):
        qblk[b, s_i, p_i, hs] = q[b] * SCALE
    qblk = np.ascontiguousarray(qblk.transpose(2, 0, 1, 3).reshape(128, B * 8 * 16))

    pk = np.asarray(past_k, np.float32).reshape(B, T, D)
    pv = np.asarray(past_v, np.float32).reshape(B, T, D)

    in_maps = []
    valid = np.zeros((NC, B), np.int64)
    for c in range(NC):
        m = {"qblk": qblk}
        for b in range(B):
            n = n_bs[b]
            lo = c * n
            hi = min((c + 1) * n, int(P[b]))
            v = max(0, hi - lo)
            valid[c, b] = v
            kc = np.zeros((n, D), np.float32)
            vc = np.zeros((n, D), np.float32)
            if v > 0:
                kc[:v] = pk[b, lo:lo + v]
                vc[:v] = pv[b, lo:lo + v]
            m[f"kt{b}"] = np.ascontiguousarray(kc.T).reshape(8, 128, n)
            m[f"v{b}"] = vc
        in_maps.append(m)

    meta = dict(q=q, k_step=k_step, v_step=v_step, n_bs=n_bs, valid=valid)
    return in_maps, meta


def _host_combine(results, meta, W_proj, b_proj):
    q, k_step, v_step = meta["q"], meta["k_step"], meta["v_step"]
    n_bs, valid = meta["n_bs"], meta["valid"]
    O = np.zeros((B, H, HD), np.float64)
    S = np.zeros((B, H), np.float64)
    hh = np.arange(H)
    nch = np.array([(n + CHUNK - 1) // CHUNK for n in n_bs])
    for c in range(NC):
        o = results[c]["o"].astype(np.float64)      # [B, 16, 1024]
        s = results[c]["s"].astype(np.float64).copy()  # [B, 16, 2]
        for b in range(B):                          # mask never-written chunk cols
            s[b, :, nch[b]:] = 0.0
        o_diag = o.reshape(B, H, H, HD)[:, hh, hh, :]  # [B, H, HD]
        pads = np.array([n_bs[b] - valid[c, b] for b in range(B)], np.float64)
        S += s.sum(axis=2) - pads[:, None]
        O += o_diag
    s_new = np.einsum("bhd,bhd->bh", q.astype(np.float64),
                      k_step.astype(np.float64)) * SCALE
    e_new = np.exp(s_new)
    S += e_new
    O += e_new[:, :, None] * v_step.astype(np.float64)
    y_attn = (O / S[:, :, None]).reshape(B, D)
    y = y_attn @ np.asarray(W_proj, np.float64) + np.asarray(b_proj, np.float64)
    return y.astype(np.float32)


def _get_built(n_bs):
    if n_bs not in _BUILD_CACHE:
        _BUILD_CACHE[n_bs] = _build_nc(n_bs)
    return _BUILD_CACHE[n_bs]


def kernel(x, past_k, past_v, idx, W_attn, b_attn, W_proj, b_proj, _trace=False):
    from concourse.bass_utils import run_bass_kernel_spmd

    in_maps, meta = _host_prep(x, past_k, past_v, idx, W_attn, b_attn)
    nc = _get_built(meta["n_bs"])
    res = run_bass_kernel_spmd(nc, in_maps, core_ids=list(range(NC)), trace=_trace)
    y = _host_combine(res.results, meta, W_proj, b_proj)
    if _trace:
        kernel._last_results = res
    return (y, meta["k_step"], meta["v_step"])


# revision 14
# speedup vs baseline: 1.3412x; 1.0679x over previous
"""Trainium2 Bass kernel for GPT2 sparse-attention decode step (B=8, T=8192, H=16, HD=64).

Contract: kernel(**inputs) takes FULL unsharded inputs, returns the FULL output
tuple (y[8,1024], k_step[8,16,64], v_step[8,16,64]) matching reference().

Strategy (memory-regime): only cache positions t <= idx[b] matter. The tiny
qkv / output projections and the flash-style combine run on host; the 8
NeuronCores do the memory-dominant sweep over the valid KV prefix. Each
batch's prefix [0, idx[b]) is split contiguously into 8 equal per-core chunks
(balanced load); the freshly-written token at t=idx[b] is folded in on host.

Per (core, batch) device program:
  scores[h,t] = sum_d q[b,h,d]*K[t,h,d]*scale  -- PE matmul with block-diag q
                (contraction over hd on partitions; K shipped pre-transposed
                 [hd, t] by host), accumulated over 8 hd-slices into PSUM.
  e = exp(scores)                              -- ScalarE, fused row-sum
  p^T via PE transpose (scores are provably small => no max-stabilization)
  o[h,:] = sum_t p[t,h] * V[t,:]               -- PE matmul, natural-layout V,
                diag 64-wide strips extracted on host.
Zero-padded tail positions contribute exp(0)=1 to the denominator and 0 to o;
the host subtracts the exact pad count.
"""
import math
import os
import sys

import numpy as np

for _p in ("/root/.axon_site/_ro/trn_rl_repo", "/opt/trn_rl_repo"):
    if os.path.isdir(_p) and _p not in sys.path:
        sys.path.append(_p)

B, T, D, H, HD = 8, 8192, 1024, 16, 64
NC = 8
SCALE = 1.0 / 8.0
CHUNK = 512
SCORES_F32R = True   # float32r matmul for q@K^T (1 cy/row vs 4 for fp32)
PV_F32R = True       # float32r matmul for p@V

_BUILD_CACHE: dict = {}


def _build_nc(n_bs):
    """Build the SPMD Bass program for per-batch per-core lengths n_bs."""
    import concourse.bacc as bacc
    import concourse.mybir as mybir
    import concourse.tile as tile
    from concourse.masks import make_identity

    f32 = mybir.dt.float32
    f32r = mybir.dt.float32r
    kdt = f32r if SCORES_F32R else f32
    vdt = f32r if PV_F32R else f32

    nc = bacc.Bacc("TRN2")

    kt_in = [nc.declare_dram_parameter(f"kt{b}", [8, 128, n_bs[b]], kdt, isOutput=False)
             for b in range(B)]
    v_in = [nc.declare_dram_parameter(f"v{b}", [n_bs[b], D], vdt, isOutput=False)
            for b in range(B)]
    q_in = nc.declare_dram_parameter("qblk", [128, B * 8 * 16], kdt, isOutput=False)
    o_out = nc.declare_dram_parameter("o", [B, 16, D], f32, isOutput=True)
    s_out = nc.declare_dram_parameter("s", [B, 16, 2], f32, isOutput=True)

    with tile.TileContext(nc) as tc:
        with (
            tc.tile_pool(name="singles", bufs=1) as singles,
            tc.tile_pool(name="kpool", bufs=3) as kpool,
            tc.tile_pool(name="vpool", bufs=3) as vpool,
            tc.tile_pool(name="epool", bufs=2) as epool,
            tc.tile_pool(name="ptpool", bufs=2) as ptpool,
            tc.tile_pool(name="spool", bufs=4) as spool,
            tc.tile_pool(name="opool", bufs=2) as opool,
            tc.tile_pool(name="psum_s", bufs=2, space="PSUM") as psum_s,
            tc.tile_pool(name="psum_t", bufs=2, space="PSUM") as psum_t,
            tc.tile_pool(name="psum_o", bufs=2, space="PSUM") as psum_o,
        ):
            qsb = singles.tile([128, B * 8 * 16], kdt)
            nc.sync.dma_start(out=qsb[:], in_=q_in[:])
            ident = singles.tile([16, 16], f32)
            make_identity(nc, ident)

            for b in range(B):
                n = n_bs[b]
                nchunks = (n + CHUNK - 1) // CHUNK
                o_ps = psum_o.tile([16, D], f32)
                for ci in range(nchunks):
                    c0 = ci * CHUNK
                    w = min(CHUNK, n - c0)
                    njt = (w + 127) // 128

                    k_sb = kpool.tile([128, 8, CHUNK], kdt, tag="k")
                    for g in range(2):
                        nc.sync.dma_start(
                            out=k_sb[:, g * 4:(g + 1) * 4, :w],
                            in_=kt_in[b][g * 4:(g + 1) * 4, :, c0:c0 + w].rearrange(
                                "s p t -> p s t"),
                        )

                    v_sb = vpool.tile([128, 4, D], vdt, tag="v")
                    jfull = w // 128
                    rem = w % 128
                    for j0 in range(jfull):
                        nc.scalar.dma_start(
                            out=v_sb[:, j0:j0 + 1, :],
                            in_=v_in[b][c0 + j0 * 128:c0 + (j0 + 1) * 128, :].rearrange(
                                "(j p) d -> p j d", p=128),
                        )
                    if rem:
                        nc.scalar.dma_start(
                            out=v_sb[:rem, jfull, :],
                            in_=v_in[b][c0 + jfull * 128:c0 + w, :],
                        )

                    sc_ps = psum_s.tile([16, CHUNK], f32)
                    for s in range(8):
                        nc.tensor.matmul(
                            sc_ps[:, :w],
                            lhsT=qsb[:, (b * 8 + s) * 16:(b * 8 + s + 1) * 16],
                            rhs=k_sb[:, s, :w],
                            start=(s == 0),
                            stop=(s == 7),
                        )

                    e_sb = epool.tile([16, CHUNK], f32, tag="e")
                    s_sb = spool.tile([16, 1], f32, tag="s")
                    nc.scalar.activation(
                        out=e_sb[:, :w],
                        in_=sc_ps[:, :w],
                        func=mybir.ActivationFunctionType.Exp,
                        accum_out=s_sb[:],
                    )
                    nc.gpsimd.dma_start(out=s_out[b, :, ci:ci + 1], in_=s_sb[:])

                    pt_ps = psum_t.tile([128, 64], f32)
                    pt_sb = ptpool.tile([128, 64], vdt, tag="pt")
                    for j in range(njt):
                        wj = min(128, w - j * 128)
                        nc.tensor.transpose(
                            out=pt_ps[:wj, j * 16:(j + 1) * 16],
                            in_=e_sb[:, j * 128:j * 128 + wj],
                            identity=ident[:],
                        )
                        nc.vector.tensor_copy(
                            pt_sb[:wj, j * 16:(j + 1) * 16],
                            pt_ps[:wj, j * 16:(j + 1) * 16],
                        )

                    for j in range(njt):
                        wj = min(128, w - j * 128)
                        first = (ci == 0 and j == 0)
                        last = (ci == nchunks - 1 and j == njt - 1)
                        for half in range(2):
                            nc.tensor.matmul(
                                o_ps[:, half * 512:(half + 1) * 512],
                                lhsT=pt_sb[:wj, j * 16:(j + 1) * 16],
                                rhs=v_sb[:wj, j, half * 512:(half + 1) * 512],
                                start=first,
                                stop=last,
                                skip_group_check=True,
                            )

                o_sb = opool.tile([16, D], f32, tag="o")
                nc.vector.tensor_copy(o_sb[:], o_ps[:])
                nc.gpsimd.dma_start(out=o_out[b], in_=o_sb[:])

    nc.compile()
    return nc


def _host_prep(x, past_k, past_v, idx, W_attn, b_attn):
    x = np.ascontiguousarray(x, np.float32)
    qkv = x @ np.asarray(W_attn, np.float32) + np.asarray(b_attn, np.float32)
    q, k_step, v_step = np.split(qkv, 3, axis=-1)
    q = q.reshape(B, H, HD)
    k_step = np.ascontiguousarray(k_step.reshape(B, H, HD))
    v_step = np.ascontiguousarray(v_step.reshape(B, H, HD))

    idx = np.asarray(idx).astype(np.int64)
    P = idx.copy()                                  # past length (t < idx)
    # per-core per-batch positions, padded to a multiple of 4 (fp32r matmul
    # requires even moving-operand free dims; pads are exact zeros, fixed up
    # in the combine)
    n_bs = tuple(int(max(4, -(-((p + NC - 1) // NC) // 4) * 4)) for p in P)

    qblk = np.zeros((B, 8, 128, 16), np.float32)
    hs, ds_ = np.meshgrid(np.arange(H), np.arange(HD), indexing="ij")
    flat = hs * HD + ds_
    s_i, p_i = flat // 128, flat % 128
    for b in range(B):
        qblk[b, s_i, p_i, hs] = q[b] * SCALE
    qblk = np.ascontiguousarray(qblk.transpose(2, 0, 1, 3).reshape(128, B * 8 * 16))

    pk = np.asarray(past_k, np.float32).reshape(B, T, D)
    pv = np.asarray(past_v, np.float32).reshape(B, T, D)

    in_maps = []
    valid = np.zeros((NC, B), np.int64)
    for c in range(NC):
        m = {"qblk": qblk}
        for b in range(B):
            n = n_bs[b]
            lo = c * n
            hi = min((c + 1) * n, int(P[b]))
            v = max(0, hi - lo)
            valid[c, b] = v
            kc = np.zeros((n, D), np.float32)
            vc = np.zeros((n, D), np.float32)
            if v > 0:
                kc[:v] = pk[b, lo:lo + v]
                vc[:v] = pv[b, lo:lo + v]
            m[f"kt{b}"] = np.ascontiguousarray(kc.T).reshape(8, 128, n)
            m[f"v{b}"] = vc
        in_maps.append(m)

    meta = dict(q=q, k_step=k_step, v_step=v_step, n_bs=n_bs, valid=valid)
    return in_maps, meta


def _host_combine(results, meta, W_proj, b_proj):
    q, k_step, v_step = meta["q"], meta["k_step"], meta["v_step"]
    n_bs, valid = meta["n_bs"], meta["valid"]
    O = np.zeros((B, H, HD), np.float64)
    S = np.zeros((B, H), np.float64)
    hh = np.arange(H)
    nch = np.array([(n + CHUNK - 1) // CHUNK for n in n_bs])
    for c in range(NC):
        o = results[c]["o"].astype(np.float64)      # [B, 16, 1024]
        s = results[c]["s"].astype(np.float64).copy()  # [B, 16, 2]
        for b in range(B):                          # mask never-written chunk cols
            s[b, :, nch[b]:] = 0.0
        o_diag = o.reshape(B, H, H, HD)[:, hh, hh, :]  # [B, H, HD]
        pads = np.array([n_bs[b] - valid[c, b] for b in range(B)], np.float64)
        S += s.sum(axis=2) - pads[:, None]
        O += o_diag
    s_new = np.einsum("bhd,bhd->bh", q.astype(np.float64),
                      k_step.astype(np.float64)) * SCALE
    e_new = np.exp(s_new)
    S += e_new
    O += e_new[:, :, None] * v_step.astype(np.float64)
    y_attn = (O / S[:, :, None]).reshape(B, D)
    y = y_attn @ np.asarray(W_proj, np.float64) + np.asarray(b_proj, np.float64)
    return y.astype(np.float32)


def _get_built(n_bs):
    if n_bs not in _BUILD_CACHE:
        _BUILD_CACHE[n_bs] = _build_nc(n_bs)
    return _BUILD_CACHE[n_bs]


def kernel(x, past_k, past_v, idx, W_attn, b_attn, W_proj, b_proj, _trace=False):
    from concourse.bass_utils import run_bass_kernel_spmd

    in_maps, meta = _host_prep(x, past_k, past_v, idx, W_attn, b_attn)
    nc = _get_built(meta["n_bs"])
    res = run_bass_kernel_spmd(nc, in_maps, core_ids=list(range(NC)), trace=_trace)
    y = _host_combine(res.results, meta, W_proj, b_proj)
    if _trace:
        kernel._last_results = res
    return (y, meta["k_step"], meta["v_step"])
